# revision 10
# baseline (speedup 1.0000x reference)
"""Trainium2 Bass kernel: causal MHA (B=2,S=2048,D=768,H=12) on 8 NeuronCores.

Sharding: core c -> batch b=c//4, j=c%4; two q-blocks (t_lo=j, t_hi=7-j) of
S/8 rows each, for causal load balance. K/V projected fully per core.
Uniform SPMD program (one NEFF for all 8 cores; per-core data differs):
block-lo uses key tiles [0, KT_LO), mask-matmul on all of them; block-hi uses
key tiles [0, KT_HI), mask-matmul on [KT_LO, KT_HI). Masked/padded logits get
-1e9 added via a (-1e9*I) @ maskT accumulate matmul, so exp -> 0 exactly.
All data f32; matmuls run as float32r. Softmax denominator comes free from 64
ones-columns appended to V's stationary operand (rows 64..127 of the PV psum).
"""
import sys
sys.path.insert(0, "/opt/trn_rl_repo")
from contextlib import ExitStack
import numpy as np

B, S, D, H, DK = 2, 2048, 768, 12, 64
_prog_cache = {}


def build(s=S, d=D):
    import concourse.bass as bass
    import concourse.mybir as mybir
    import concourse.tile as tile
    from concourse import bacc
    from concourse.masks import make_identity

    f32, f32r = mybir.dt.float32, mybir.dt.float32r
    P = 128
    nck = d // P              # D chunks (6)
    qb = s // 8               # q rows per block (256)
    kt_lo, kt_hi = s // 2 // P, s // P   # 8, 16
    nheads = d // 64
    scale = 1.0 / float(np.sqrt(d))
    Exp = mybir.ActivationFunctionType.Exp
    Relu = mybir.ActivationFunctionType.Relu

    nc = bacc.Bacc("TRN2", target_bir_lowering=False, debug=False)
    with tile.TileContext(nc) as tc, ExitStack() as top:
        dram = top.enter_context(tc.tile_pool(name="dram", bufs=1, space="DRAM"))
        xq = dram.tile([2 * qb, d], f32, kind="ExternalInput")
        xk = dram.tile([s, d], f32, kind="ExternalInput")
        xv = dram.tile([s, d], f32, kind="ExternalInput")
        mT = dram.tile([kt_hi, P, 2 * qb], f32, kind="ExternalInput")
        Wqd = dram.tile([d, d], f32, kind="ExternalInput")
        Wkd = dram.tile([d, d], f32, kind="ExternalInput")
        Wvd = dram.tile([d, d], f32, kind="ExternalInput")
        Wod = dram.tile([d, d], f32, kind="ExternalInput")
        bqd = dram.tile([nck, P], f32, kind="ExternalInput")
        bkd = dram.tile([nck, P], f32, kind="ExternalInput")
        bvd = dram.tile([nck, P], f32, kind="ExternalInput")
        bod = dram.tile([1, d], f32, kind="ExternalInput")
        out = dram.tile([2 * qb, d], f32, kind="ExternalOutput")

        persist = top.enter_context(tc.tile_pool(name="persist", bufs=1))
        KT = persist.tile([P, nck, s], f32)
        VA = persist.tile([P, s // P, d + 64], f32)
        QT = persist.tile([P, nck, 2 * qb], f32)
        AT = persist.tile([P, nck, 2 * qb], f32)
        ident = persist.tile([P, P], f32)
        negI = persist.tile([P, P], f32)
        biasq = persist.tile([P, nck], f32)
        biask = persist.tile([P, nck], f32)
        bvc_sb = persist.tile([P, nck], f32)
        bo_sb = persist.tile([1, d], f32)
        boP = persist.tile([1, d], f32)
        ones1 = persist.tile([1, P], f32)

        make_identity(nc, ident)
        ones_st = persist.tile([P, 64], f32)
        nc.scalar.mul(negI[:].bitcast(f32r), ident, -1e9)
        nc.vector.memset(ones_st, 1.0)
        ones1_st = persist.tile([1, P], f32)
        nc.vector.memset(ones1_st, 1.0)
        nc.vector.tensor_copy(ones1[:].bitcast(f32r), ones1_st)
        for _kt in range(s // P):
            nc.vector.tensor_copy(VA[:, _kt, d:d + 64].bitcast(f32r), ones_st)
        nc.sync.dma_start(biasq, bqd[:].rearrange("a b -> b a"))
        nc.sync.dma_start(biask, bkd[:].rearrange("a b -> b a"))
        nc.sync.dma_start(bvc_sb[:].bitcast(f32r), bvd[:].rearrange("a b -> b a").bitcast(f32r))
        nc.sync.dma_start(bo_sb, bod)

        def r32(ap):
            return ap.bitcast(f32r)

        def nsplits(n):
            return [(i * 512, min(512, n - i * 512)) for i in range((n + 511) // 512)]

        def make_load_xT(stage, xtp, pt):
            def load_xT(xdram, row0, nrows):
                xT = xtp.tile([P, nck, nrows], f32, tag="xT")
                for sc in range(nrows // P):
                    xn = stage.tile([P, d], f32, tag="xn")
                    nc.sync.dma_start(xn, xdram[row0 + sc * P:row0 + (sc + 1) * P, :])
                    for dc in range(nck):
                        tp = pt.tile([P, P], f32, tag="tp")
                        nc.tensor.transpose(tp, xn[:, dc * P:(dc + 1) * P], ident)
                        nc.vector.tensor_copy(xT[:, dc, sc * P:(sc + 1) * P].bitcast(f32r), tp)
                return xT
            return load_xT

        with ExitStack() as ph2a:
            wqpool = ph2a.enter_context(tc.tile_pool(name="wqpool", bufs=1))
            stage = ph2a.enter_context(tc.tile_pool(name="stageq", bufs=3))
            xtp = ph2a.enter_context(tc.tile_pool(name="xtpq", bufs=1))
            pp = ph2a.enter_context(tc.tile_pool(name="ppq", bufs=3, space="PSUM"))
            pt = ph2a.enter_context(tc.tile_pool(name="ptq", bufs=3, space="PSUM"))
            load_xT = make_load_xT(stage, xtp, pt)
            Wq_sb = wqpool.tile([P, nck, d], f32, tag="wq")
            nc.sync.dma_start(Wq_sb[:].bitcast(f32r), Wqd[:].rearrange("(c p) n -> p c n", p=P).bitcast(f32r))
            xqT = load_xT(xq, 0, 2 * qb)
            for dc in range(nck):
                ps = pp.tile([P, 512], f32, tag="ps")
                for kc in range(nck):
                    nc.tensor.matmul(ps[:, :2 * qb],
                                     r32(Wq_sb[:, kc, dc * P:(dc + 1) * P]),
                                     r32(xqT[:, kc, :]),
                                     start=(kc == 0), stop=(kc == nck - 1))
                nc.vector.tensor_scalar_add(QT[:, dc, :].bitcast(f32r), ps[:, :2 * qb],
                                            biasq[:, dc:dc + 1])

        with ExitStack() as ph2b:
            wpool = ph2b.enter_context(tc.tile_pool(name="wpool", bufs=1))
            stage = ph2b.enter_context(tc.tile_pool(name="stage", bufs=3))
            xtp = ph2b.enter_context(tc.tile_pool(name="xtp", bufs=1))
            pp = ph2b.enter_context(tc.tile_pool(name="pp", bufs=3, space="PSUM"))
            pt = ph2b.enter_context(tc.tile_pool(name="pt", bufs=3, space="PSUM"))
            load_xT = make_load_xT(stage, xtp, pt)
            Wk_sb = wpool.tile([P, nck, d], f32, tag="wk")
            Wv_sb = wpool.tile([P, nck, d], f32, tag="wv")
            nc.sync.dma_start(Wk_sb[:].bitcast(f32r), Wkd[:].rearrange("(c p) n -> p c n", p=P).bitcast(f32r))
            nc.sync.dma_start(Wv_sb[:].bitcast(f32r), Wvd[:].rearrange("(c p) n -> p c n", p=P).bitcast(f32r))
            for g in range(s // 512):
                xkT = load_xT(xk, g * 512, 512)
                for dc in range(nck):
                    ps = pp.tile([P, 512], f32, tag="ps")
                    for kc in range(nck):
                        nc.tensor.matmul(ps, r32(Wk_sb[:, kc, dc * P:(dc + 1) * P]),
                                         r32(xkT[:, kc, :]),
                                         start=(kc == 0), stop=(kc == nck - 1))
                    nc.vector.tensor_scalar_add(KT[:, dc, g * 512:(g + 1) * 512].bitcast(f32r),
                                                ps, biask[:, dc:dc + 1])
                xvT = load_xT(xv, g * 512, 512)
                for sc in range(4):
                    kt = g * 4 + sc
                    for n0, nn in nsplits(d):
                        ps = pp.tile([P, 512], f32, tag="ps")
                        for kc in range(nck):
                            nc.tensor.matmul(ps[:, :nn],
                                             r32(xvT[:, kc, sc * P:(sc + 1) * P]),
                                             r32(Wv_sb[:, kc, n0:n0 + nn]),
                                             start=(kc == 0), stop=(kc == nck - 1))
                        nc.vector.tensor_copy(VA[:, kt, n0:n0 + nn].bitcast(f32r), ps[:, :nn])

        # ---- attention ----
        import concourse.bass as bass_mod
        with ExitStack() as ph3:
            mpool = ph3.enter_context(tc.tile_pool(name="mpool", bufs=1))
            epool = ph3.enter_context(tc.tile_pool(name="epool", bufs=4))
            rpool = ph3.enter_context(tc.tile_pool(name="rpool", bufs=3))
            lps = ph3.enter_context(tc.tile_pool(name="lps", bufs=3, space="PSUM"))
            aps = ph3.enter_context(tc.tile_pool(name="aps", bufs=2, space="PSUM"))
            mTs = mpool.tile([P, kt_hi, 2 * qb], f32)
            nc.sync.dma_start(mTs[:].bitcast(f32r), mT[:].rearrange("t p c -> p t c").bitcast(f32r))

            for blk, nkt, m0 in ((0, kt_lo, 0), (1, kt_hi, kt_lo)):
                q0 = blk * qb
                for h in range(nheads):
                    hp, hc = (h % 2) * 64, h // 2
                    ap_psum = aps.tile([64, qb], f32, tag="apv")
                    den_psum = aps.tile([64, qb], f32, tag="den")
                    for kt in range(nkt):
                        lg = lps.tile([P, qb], f32, tag="lg")
                        nc.tensor.matmul(
                            lg, r32(KT[hp:hp + 64, hc, kt * P:(kt + 1) * P]),
                            r32(QT[hp:hp + 64, hc, q0:q0 + qb]),
                            start=True, stop=(kt < m0))
                        if kt >= m0:
                            nc.tensor.matmul(lg, r32(negI),
                                             r32(mTs[:, kt, q0:q0 + qb]),
                                             start=False, stop=True)
                        E = epool.tile([P, qb], f32, tag="E")
                        nc.scalar.activation(E[:].bitcast(f32r), lg, Exp, scale=scale)
                        nc.tensor.matmul(ap_psum,
                                         r32(VA[:, kt, h * 64:(h + 1) * 64]),
                                         r32(E[:]),
                                         start=(kt == 0), stop=(kt == nkt - 1))
                        nc.tensor.matmul(den_psum,
                                         r32(VA[:, kt, d:d + 64]),
                                         r32(E[:]),
                                         start=(kt == 0), stop=(kt == nkt - 1))
                    rec = rpool.tile([64, qb], f32, tag="rec")
                    nc.vector.reciprocal(rec, den_psum)
                    nc.vector.tensor_mul(AT[hp:hp + 64, hc, q0:q0 + qb].bitcast(f32r),
                                         ap_psum, rec)

        # ---- O-projection + bo' + relu ----
        with ExitStack() as ph4:
            wo_pool = ph4.enter_context(tc.tile_pool(name="wo", bufs=1))
            opool = ph4.enter_context(tc.tile_pool(name="opool", bufs=2))
            ops = ph4.enter_context(tc.tile_pool(name="ops", bufs=2, space="PSUM"))
            Wo_sb = wo_pool.tile([P, nck, d], f32)
            nc.sync.dma_start(Wo_sb[:].bitcast(f32r), Wod[:].rearrange("(c p) n -> p c n", p=P).bitcast(f32r))
            # bo' = bv @ Wo + bo
            for n0, nn in nsplits(d):
                ps = ops.tile([P, 512], f32, tag="pso")
                for kc in range(nck):
                    nc.tensor.matmul(ps[:1, :nn], r32(bvc_sb[:, kc:kc + 1]),
                                     r32(Wo_sb[:, kc, n0:n0 + nn]),
                                     start=(kc == 0), stop=(kc == nck - 1))
                nc.vector.tensor_add(boP[:, n0:n0 + nn].bitcast(f32r), ps[:1, :nn],
                                     bo_sb[:, n0:n0 + nn])
            for sub in range(2 * qb // P):
                osb = opool.tile([P, d], f32, tag="osb")
                for n0, nn in nsplits(d):
                    ps = ops.tile([P, 512], f32, tag="pso")
                    for kc in range(nck):
                        nc.tensor.matmul(ps[:, :nn],
                                         r32(AT[:, kc, sub * P:(sub + 1) * P]),
                                         r32(Wo_sb[:, kc, n0:n0 + nn]),
                                         start=(kc == 0), stop=False)
                    nc.tensor.matmul(ps[:, :nn], r32(ones1),
                                     r32(boP[:, n0:n0 + nn]),
                                     start=False, stop=True)
                    nc.scalar.activation(osb[:, n0:n0 + nn], ps[:, :nn], Relu)
                nc.sync.dma_start(out[sub * P:(sub + 1) * P, :], osb)

    nc.compile()
    names = dict(xq=xq.name, xk=xk.name, xv=xv.name, mT=mT.name,
                 Wq=Wqd.name, Wk=Wkd.name, Wv=Wvd.name, Wo=Wod.name,
                 bq=bqd.name, bk=bkd.name, bv=bvd.name, bo=bod.name,
                 out=out.name)
    return nc, names


def make_in_maps(names, q, k, v, mask, Wq, bq, Wk, bk, Wv, bv, Wo, bo,
                 s=S, d=D, n_cores=8):
    qb = s // 8
    kt_lo, kt_hi = s // 2 // 128, s // 128
    nck = d // 128
    mask2d = np.asarray(mask, np.float32).reshape(s, s)
    f = lambda x: np.ascontiguousarray(np.asarray(x), dtype=np.float32)
    in_maps = []
    for c in range(n_cores):
        b, j = c // 4, c % 4
        lo = slice(j * qb, (j + 1) * qb)
        hi = slice((7 - j) * qb, (8 - j) * qb)
        mTc = np.zeros((kt_hi, 128, 2 * qb), np.float32)
        for kt in range(kt_lo):
            mTc[kt, :, 0:qb] = mask2d[lo, kt * 128:(kt + 1) * 128].T
        for kt in range(kt_lo, kt_hi):
            mTc[kt, :, qb:2 * qb] = mask2d[hi, kt * 128:(kt + 1) * 128].T
        in_maps.append({
            names["xq"]: np.concatenate([f(q[b])[lo], f(q[b])[hi]], 0),
            names["xk"]: f(k[b]), names["xv"]: f(v[b]), names["mT"]: mTc,
            names["Wq"]: f(Wq), names["Wk"]: f(Wk), names["Wv"]: f(Wv),
            names["Wo"]: f(Wo),
            names["bq"]: f(bq).reshape(nck, 128),
            names["bk"]: f(bk).reshape(nck, 128),
            names["bv"]: f(bv).reshape(nck, 128),
            names["bo"]: f(bo).reshape(1, d),
        })
    return in_maps


def unshard(results, out_name, s=S, d=D):
    qb = s // 8
    full = np.zeros((B, s, d), np.float32)
    for c in range(len(results)):
        b, j = c // 4, c % 4
        oc = results[c][out_name]
        full[b, j * qb:(j + 1) * qb] = oc[:qb]
        full[b, (7 - j) * qb:(8 - j) * qb] = oc[qb:]
    return full


def kernel(q, k, v, mask, Wq, bq, Wk, bk, Wv, bv, Wo, bo):
    from concourse.bass_utils import run_bass_kernel_spmd
    if "prog" not in _prog_cache:
        _prog_cache["prog"] = build()
    nc, names = _prog_cache["prog"]
    in_maps = make_in_maps(names, q, k, v, mask, Wq, bq, Wk, bk, Wv, bv, Wo, bo)
    res = run_bass_kernel_spmd(nc, in_maps, core_ids=list(range(8)))
    return unshard(res.results, names["out"])


# revision 11
# speedup vs baseline: 1.0081x; 1.0081x over previous
"""Trainium2 Bass kernel: causal MHA (B=2,S=2048,D=768,H=12) on 8 NeuronCores.

Sharding: core c -> batch b=c//4, j=c%4; two q-blocks (t_lo=j, t_hi=7-j) of
S/8 rows each, for causal load balance. K/V projected fully per core.
Uniform SPMD program (one NEFF for all 8 cores; per-core data differs):
block-lo uses key tiles [0, KT_LO), mask-matmul on all of them; block-hi uses
key tiles [0, KT_HI), mask-matmul on [KT_LO, KT_HI). Masked/padded logits get
-1e9 added via a (-1e9*I) @ maskT accumulate matmul, so exp -> 0 exactly.
All data f32; matmuls run as float32r. Softmax denominator comes free from 64
ones-columns appended to V's stationary operand (rows 64..127 of the PV psum).
"""
import sys
sys.path.insert(0, "/opt/trn_rl_repo")
from contextlib import ExitStack
import numpy as np

B, S, D, H, DK = 2, 2048, 768, 12, 64
_prog_cache = {}


def build(s=S, d=D):
    import concourse.bass as bass
    import concourse.mybir as mybir
    import concourse.tile as tile
    from concourse import bacc
    from concourse.masks import make_identity

    f32, f32r = mybir.dt.float32, mybir.dt.float32r
    P = 128
    nck = d // P              # D chunks (6)
    qb = s // 8               # q rows per block (256)
    kt_lo, kt_hi = s // 2 // P, s // P   # 8, 16
    nheads = d // 64
    scale = 1.0 / float(np.sqrt(d))
    Exp = mybir.ActivationFunctionType.Exp
    Relu = mybir.ActivationFunctionType.Relu

    nc = bacc.Bacc("TRN2", target_bir_lowering=False, debug=False)
    with tile.TileContext(nc) as tc, ExitStack() as top:
        dram = top.enter_context(tc.tile_pool(name="dram", bufs=1, space="DRAM"))
        xq = dram.tile([2 * qb, d], f32, kind="ExternalInput")
        xk = dram.tile([s, d], f32, kind="ExternalInput")
        xv = dram.tile([s, d], f32, kind="ExternalInput")
        mT = dram.tile([kt_hi, P, 2 * qb], f32, kind="ExternalInput")
        Wqd = dram.tile([d, d], f32, kind="ExternalInput")
        Wkd = dram.tile([d, d], f32, kind="ExternalInput")
        Wvd = dram.tile([d, d], f32, kind="ExternalInput")
        Wod = dram.tile([d, d], f32, kind="ExternalInput")
        bqd = dram.tile([nck, P], f32, kind="ExternalInput")
        bkd = dram.tile([nck, P], f32, kind="ExternalInput")
        bvd = dram.tile([nck, P], f32, kind="ExternalInput")
        bod = dram.tile([1, d], f32, kind="ExternalInput")
        out = dram.tile([2 * qb, d], f32, kind="ExternalOutput")

        persist = top.enter_context(tc.tile_pool(name="persist", bufs=1))
        KT = persist.tile([P, nck, s], f32)
        VA = persist.tile([P, s // P, d], f32)
        ones64 = persist.tile([P, 64], f32)
        QT = persist.tile([P, nck, 2 * qb], f32)
        AT = persist.tile([P, nck, 2 * qb], f32)
        ident = persist.tile([P, P], f32)
        negI = persist.tile([P, P], f32)
        biasq = persist.tile([P, nck], f32)
        biask = persist.tile([P, nck], f32)
        bvc_sb = persist.tile([P, nck], f32)
        bo_sb = persist.tile([1, d], f32)
        boP = persist.tile([1, d], f32)
        ones1 = persist.tile([1, P], f32)

        make_identity(nc, ident)
        ones_st = persist.tile([P, 64], f32)
        nc.scalar.mul(negI[:].bitcast(f32r), ident, -1e9)
        nc.vector.memset(ones_st, 1.0)
        ones1_st = persist.tile([1, P], f32)
        nc.vector.memset(ones1_st, 1.0)
        nc.vector.tensor_copy(ones1[:].bitcast(f32r), ones1_st)
        nc.vector.tensor_copy(ones64[:].bitcast(f32r), ones_st)
        nc.sync.dma_start(biasq, bqd[:].rearrange("a b -> b a"))
        nc.sync.dma_start(biask, bkd[:].rearrange("a b -> b a"))
        nc.sync.dma_start(bvc_sb[:].bitcast(f32r), bvd[:].rearrange("a b -> b a").bitcast(f32r))
        nc.sync.dma_start(bo_sb, bod)

        def r32(ap):
            return ap.bitcast(f32r)

        def nsplits(n):
            return [(i * 512, min(512, n - i * 512)) for i in range((n + 511) // 512)]

        def make_load_xT(stage, xtp, pt):
            def load_xT(xdram, row0, nrows):
                xT = xtp.tile([P, nck, nrows], f32, tag="xT")
                for sc in range(nrows // P):
                    xn = stage.tile([P, d], f32, tag="xn")
                    nc.sync.dma_start(xn, xdram[row0 + sc * P:row0 + (sc + 1) * P, :])
                    for dc in range(nck):
                        tp = pt.tile([P, P], f32, tag="tp")
                        nc.tensor.transpose(tp, xn[:, dc * P:(dc + 1) * P], ident)
                        nc.vector.tensor_copy(xT[:, dc, sc * P:(sc + 1) * P].bitcast(f32r), tp)
                return xT
            return load_xT

        with ExitStack() as ph2a:
            wqpool = ph2a.enter_context(tc.tile_pool(name="wqpool", bufs=1))
            stage = ph2a.enter_context(tc.tile_pool(name="stageq", bufs=3))
            xtp = ph2a.enter_context(tc.tile_pool(name="xtpq", bufs=2))
            pp = ph2a.enter_context(tc.tile_pool(name="ppq", bufs=3, space="PSUM"))
            pt = ph2a.enter_context(tc.tile_pool(name="ptq", bufs=3, space="PSUM"))
            load_xT = make_load_xT(stage, xtp, pt)
            Wq_sb = wqpool.tile([P, nck, d], f32, tag="wq")
            nc.sync.dma_start(Wq_sb[:].bitcast(f32r), Wqd[:].rearrange("(c p) n -> p c n", p=P).bitcast(f32r))
            xqT = load_xT(xq, 0, 2 * qb)
            for dc in range(nck):
                ps = pp.tile([P, 512], f32, tag="ps")
                for kc in range(nck):
                    nc.tensor.matmul(ps[:, :2 * qb],
                                     r32(Wq_sb[:, kc, dc * P:(dc + 1) * P]),
                                     r32(xqT[:, kc, :]),
                                     start=(kc == 0), stop=(kc == nck - 1))
                nc.vector.tensor_scalar_add(QT[:, dc, :].bitcast(f32r), ps[:, :2 * qb],
                                            biasq[:, dc:dc + 1])

        with ExitStack() as ph2b:
            wpool = ph2b.enter_context(tc.tile_pool(name="wpool", bufs=1))
            stage = ph2b.enter_context(tc.tile_pool(name="stage", bufs=3))
            xtp = ph2b.enter_context(tc.tile_pool(name="xtp", bufs=2))
            pp = ph2b.enter_context(tc.tile_pool(name="pp", bufs=3, space="PSUM"))
            pt = ph2b.enter_context(tc.tile_pool(name="pt", bufs=3, space="PSUM"))
            load_xT = make_load_xT(stage, xtp, pt)
            Wk_sb = wpool.tile([P, nck, d], f32, tag="wk")
            Wv_sb = wpool.tile([P, nck, d], f32, tag="wv")
            nc.sync.dma_start(Wk_sb[:].bitcast(f32r), Wkd[:].rearrange("(c p) n -> p c n", p=P).bitcast(f32r))
            nc.sync.dma_start(Wv_sb[:].bitcast(f32r), Wvd[:].rearrange("(c p) n -> p c n", p=P).bitcast(f32r))
            for g in range(s // 512):
                xkT = load_xT(xk, g * 512, 512)
                for dc in range(nck):
                    ps = pp.tile([P, 512], f32, tag="ps")
                    for kc in range(nck):
                        nc.tensor.matmul(ps, r32(Wk_sb[:, kc, dc * P:(dc + 1) * P]),
                                         r32(xkT[:, kc, :]),
                                         start=(kc == 0), stop=(kc == nck - 1))
                    nc.vector.tensor_scalar_add(KT[:, dc, g * 512:(g + 1) * 512].bitcast(f32r),
                                                ps, biask[:, dc:dc + 1])
                xvT = load_xT(xv, g * 512, 512)
                for sc in range(4):
                    kt = g * 4 + sc
                    for n0, nn in nsplits(d):
                        ps = pp.tile([P, 512], f32, tag="ps")
                        for kc in range(nck):
                            nc.tensor.matmul(ps[:, :nn],
                                             r32(xvT[:, kc, sc * P:(sc + 1) * P]),
                                             r32(Wv_sb[:, kc, n0:n0 + nn]),
                                             start=(kc == 0), stop=(kc == nck - 1))
                        nc.vector.tensor_copy(VA[:, kt, n0:n0 + nn].bitcast(f32r), ps[:, :nn])

        # ---- attention ----
        import concourse.bass as bass_mod
        with ExitStack() as ph3:
            mpool = ph3.enter_context(tc.tile_pool(name="mpool", bufs=1))
            epool = ph3.enter_context(tc.tile_pool(name="epool", bufs=4))
            rpool = ph3.enter_context(tc.tile_pool(name="rpool", bufs=3))
            lps = ph3.enter_context(tc.tile_pool(name="lps", bufs=3, space="PSUM"))
            aps = ph3.enter_context(tc.tile_pool(name="aps", bufs=2, space="PSUM"))
            mTs = mpool.tile([P, kt_hi, 2 * qb], f32)
            nc.sync.dma_start(mTs[:].bitcast(f32r), mT[:].rearrange("t p c -> p t c").bitcast(f32r))

            for blk, nkt, m0 in ((0, kt_lo, 0), (1, kt_hi, kt_lo)):
                q0 = blk * qb
                for h in range(nheads):
                    hp, hc = (h % 2) * 64, h // 2
                    ap_psum = aps.tile([64, qb], f32, tag="apv")
                    den_psum = aps.tile([64, qb], f32, tag="den")
                    for kt in range(nkt):
                        lg = lps.tile([P, qb], f32, tag="lg")
                        nc.tensor.matmul(
                            lg, r32(KT[hp:hp + 64, hc, kt * P:(kt + 1) * P]),
                            r32(QT[hp:hp + 64, hc, q0:q0 + qb]),
                            start=True, stop=(kt < m0))
                        if kt >= m0:
                            nc.tensor.matmul(lg, r32(negI),
                                             r32(mTs[:, kt, q0:q0 + qb]),
                                             start=False, stop=True)
                        E = epool.tile([P, qb], f32, tag="E")
                        nc.scalar.activation(E[:].bitcast(f32r), lg, Exp, scale=scale)
                        nc.tensor.matmul(ap_psum,
                                         r32(VA[:, kt, h * 64:(h + 1) * 64]),
                                         r32(E[:]),
                                         start=(kt == 0), stop=(kt == nkt - 1))
                        nc.tensor.matmul(den_psum,
                                         r32(ones64[:]),
                                         r32(E[:]),
                                         start=(kt == 0), stop=(kt == nkt - 1))
                    rec = rpool.tile([64, qb], f32, tag="rec")
                    nc.vector.reciprocal(rec, den_psum)
                    nc.vector.tensor_mul(AT[hp:hp + 64, hc, q0:q0 + qb].bitcast(f32r),
                                         ap_psum, rec)

        # ---- O-projection + bo' + relu ----
        with ExitStack() as ph4:
            wo_pool = ph4.enter_context(tc.tile_pool(name="wo", bufs=1))
            opool = ph4.enter_context(tc.tile_pool(name="opool", bufs=2))
            ops = ph4.enter_context(tc.tile_pool(name="ops", bufs=2, space="PSUM"))
            Wo_sb = wo_pool.tile([P, nck, d], f32)
            nc.sync.dma_start(Wo_sb[:].bitcast(f32r), Wod[:].rearrange("(c p) n -> p c n", p=P).bitcast(f32r))
            # bo' = bv @ Wo + bo
            for n0, nn in nsplits(d):
                ps = ops.tile([P, 512], f32, tag="pso")
                for kc in range(nck):
                    nc.tensor.matmul(ps[:1, :nn], r32(bvc_sb[:, kc:kc + 1]),
                                     r32(Wo_sb[:, kc, n0:n0 + nn]),
                                     start=(kc == 0), stop=(kc == nck - 1))
                nc.vector.tensor_add(boP[:, n0:n0 + nn].bitcast(f32r), ps[:1, :nn],
                                     bo_sb[:, n0:n0 + nn])
            for sub in range(2 * qb // P):
                osb = opool.tile([P, d], f32, tag="osb")
                for n0, nn in nsplits(d):
                    ps = ops.tile([P, 512], f32, tag="pso")
                    for kc in range(nck):
                        nc.tensor.matmul(ps[:, :nn],
                                         r32(AT[:, kc, sub * P:(sub + 1) * P]),
                                         r32(Wo_sb[:, kc, n0:n0 + nn]),
                                         start=(kc == 0), stop=False)
                    nc.tensor.matmul(ps[:, :nn], r32(ones1),
                                     r32(boP[:, n0:n0 + nn]),
                                     start=False, stop=True)
                    nc.scalar.activation(osb[:, n0:n0 + nn], ps[:, :nn], Relu)
                nc.sync.dma_start(out[sub * P:(sub + 1) * P, :], osb)

    nc.compile()
    names = dict(xq=xq.name, xk=xk.name, xv=xv.name, mT=mT.name,
                 Wq=Wqd.name, Wk=Wkd.name, Wv=Wvd.name, Wo=Wod.name,
                 bq=bqd.name, bk=bkd.name, bv=bvd.name, bo=bod.name,
                 out=out.name)
    return nc, names


def make_in_maps(names, q, k, v, mask, Wq, bq, Wk, bk, Wv, bv, Wo, bo,
                 s=S, d=D, n_cores=8):
    qb = s // 8
    kt_lo, kt_hi = s // 2 // 128, s // 128
    nck = d // 128
    mask2d = np.asarray(mask, np.float32).reshape(s, s)
    f = lambda x: np.ascontiguousarray(np.asarray(x), dtype=np.float32)
    in_maps = []
    for c in range(n_cores):
        b, j = c // 4, c % 4
        lo = slice(j * qb, (j + 1) * qb)
        hi = slice((7 - j) * qb, (8 - j) * qb)
        mTc = np.zeros((kt_hi, 128, 2 * qb), np.float32)
        for kt in range(kt_lo):
            mTc[kt, :, 0:qb] = mask2d[lo, kt * 128:(kt + 1) * 128].T
        for kt in range(kt_lo, kt_hi):
            mTc[kt, :, qb:2 * qb] = mask2d[hi, kt * 128:(kt + 1) * 128].T
        in_maps.append({
            names["xq"]: np.concatenate([f(q[b])[lo], f(q[b])[hi]], 0),
            names["xk"]: f(k[b]), names["xv"]: f(v[b]), names["mT"]: mTc,
            names["Wq"]: f(Wq), names["Wk"]: f(Wk), names["Wv"]: f(Wv),
            names["Wo"]: f(Wo),
            names["bq"]: f(bq).reshape(nck, 128),
            names["bk"]: f(bk).reshape(nck, 128),
            names["bv"]: f(bv).reshape(nck, 128),
            names["bo"]: f(bo).reshape(1, d),
        })
    return in_maps


def unshard(results, out_name, s=S, d=D):
    qb = s // 8
    full = np.zeros((B, s, d), np.float32)
    for c in range(len(results)):
        b, j = c // 4, c % 4
        oc = results[c][out_name]
        full[b, j * qb:(j + 1) * qb] = oc[:qb]
        full[b, (7 - j) * qb:(8 - j) * qb] = oc[qb:]
    return full


def kernel(q, k, v, mask, Wq, bq, Wk, bk, Wv, bv, Wo, bo):
    from concourse.bass_utils import run_bass_kernel_spmd
    if "prog" not in _prog_cache:
        _prog_cache["prog"] = build()
    nc, names = _prog_cache["prog"]
    in_maps = make_in_maps(names, q, k, v, mask, Wq, bq, Wk, bk, Wv, bv, Wo, bo)
    res = run_bass_kernel_spmd(nc, in_maps, core_ids=list(range(8)))
    return unshard(res.results, names["out"])


# revision 14
# speedup vs baseline: 1.0152x; 1.0071x over previous
"""Trainium2 Bass kernel: causal MHA (B=2,S=2048,D=768,H=12) on 8 NeuronCores.

Sharding: core c -> batch b=c//4, j=c%4; two q-blocks (t_lo=j, t_hi=7-j) of
S/8 rows each, for causal load balance. K/V projected fully per core.
Uniform SPMD program (one NEFF for all 8 cores; per-core data differs):
block-lo uses key tiles [0, KT_LO), mask-matmul on all of them; block-hi uses
key tiles [0, KT_HI), mask-matmul on [KT_LO, KT_HI). Masked/padded logits get
-1e9 added via a (-1e9*I) @ maskT accumulate matmul, so exp -> 0 exactly.
All data f32; matmuls run as float32r. Softmax denominator accumulates in its
own PSUM tile via a shared ones[128,64] stationary operand alongside the PV
matmuls; normalization is a per-partition DVE reciprocal+multiply.
"""
import sys
sys.path.insert(0, "/opt/trn_rl_repo")
from contextlib import ExitStack
import numpy as np

B, S, D, H, DK = 2, 2048, 768, 12, 64
_prog_cache = {}


def build(s=S, d=D):
    import concourse.bass as bass
    import concourse.mybir as mybir
    import concourse.tile as tile
    from concourse import bacc
    from concourse.masks import make_identity

    f32, f32r = mybir.dt.float32, mybir.dt.float32r
    P = 128
    nck = d // P              # D chunks (6)
    qb = s // 8               # q rows per block (256)
    kt_lo, kt_hi = s // 2 // P, s // P   # 8, 16
    nheads = d // 64
    scale = 1.0 / float(np.sqrt(d))
    Exp = mybir.ActivationFunctionType.Exp
    Relu = mybir.ActivationFunctionType.Relu

    nc = bacc.Bacc("TRN2", target_bir_lowering=False, debug=False)
    with tile.TileContext(nc) as tc, ExitStack() as top:
        dram = top.enter_context(tc.tile_pool(name="dram", bufs=1, space="DRAM"))
        xq = dram.tile([2 * qb, d], f32, kind="ExternalInput")
        xk = dram.tile([s, d], f32, kind="ExternalInput")
        xv = dram.tile([s, d], f32, kind="ExternalInput")
        mT = dram.tile([kt_hi, P, 2 * qb], f32, kind="ExternalInput")
        Wqd = dram.tile([d, d], f32, kind="ExternalInput")
        Wkd = dram.tile([d, d], f32, kind="ExternalInput")
        Wvd = dram.tile([d, d], f32, kind="ExternalInput")
        Wod = dram.tile([d, d], f32, kind="ExternalInput")
        bqd = dram.tile([nck, P], f32, kind="ExternalInput")
        bkd = dram.tile([nck, P], f32, kind="ExternalInput")
        bvd = dram.tile([nck, P], f32, kind="ExternalInput")
        bod = dram.tile([1, d], f32, kind="ExternalInput")
        out = dram.tile([2 * qb, d], f32, kind="ExternalOutput")

        persist = top.enter_context(tc.tile_pool(name="persist", bufs=1))
        KT = persist.tile([P, nck, s], f32)
        VA = persist.tile([P, s // P, d], f32)
        ones64 = persist.tile([P, 64], f32)
        QT = persist.tile([P, nck, 2 * qb], f32)
        AT = persist.tile([P, nck, 2 * qb], f32)
        ident = persist.tile([P, P], f32)
        negI = persist.tile([P, P], f32)
        biasq = persist.tile([P, nck], f32)
        biask = persist.tile([P, nck], f32)
        bvc_sb = persist.tile([P, nck], f32)
        bo_sb = persist.tile([1, d], f32)
        boP = persist.tile([1, d], f32)
        ones1 = persist.tile([1, P], f32)

        make_identity(nc, ident)
        ones_st = persist.tile([P, 64], f32)
        nc.scalar.mul(negI[:].bitcast(f32r), ident, -1e9)
        nc.vector.memset(ones_st, 1.0)
        ones1_st = persist.tile([1, P], f32)
        nc.vector.memset(ones1_st, 1.0)
        nc.vector.tensor_copy(ones1[:].bitcast(f32r), ones1_st)
        nc.vector.tensor_copy(ones64[:].bitcast(f32r), ones_st)
        nc.sync.dma_start(biasq, bqd[:].rearrange("a b -> b a"))
        nc.sync.dma_start(biask, bkd[:].rearrange("a b -> b a"))
        nc.sync.dma_start(bvc_sb[:].bitcast(f32r), bvd[:].rearrange("a b -> b a").bitcast(f32r))
        nc.sync.dma_start(bo_sb, bod)

        def r32(ap):
            return ap.bitcast(f32r)

        def nsplits(n):
            return [(i * 512, min(512, n - i * 512)) for i in range((n + 511) // 512)]

        def make_load_xT(stage, xtp, pt):
            def load_xT(xdram, row0, nrows):
                xT = xtp.tile([P, nck, nrows], f32, tag="xT")
                for sc in range(nrows // P):
                    xn = stage.tile([P, d], f32, tag="xn")
                    nc.sync.dma_start(xn, xdram[row0 + sc * P:row0 + (sc + 1) * P, :])
                    for dc in range(nck):
                        tp = pt.tile([P, P], f32, tag="tp")
                        nc.tensor.transpose(tp, xn[:, dc * P:(dc + 1) * P], ident)
                        nc.vector.tensor_copy(xT[:, dc, sc * P:(sc + 1) * P].bitcast(f32r), tp)
                return xT
            return load_xT

        with ExitStack() as ph2a:
            wqpool = ph2a.enter_context(tc.tile_pool(name="wqpool", bufs=1))
            stage = ph2a.enter_context(tc.tile_pool(name="stageq", bufs=3))
            xtp = ph2a.enter_context(tc.tile_pool(name="xtpq", bufs=2))
            pp = ph2a.enter_context(tc.tile_pool(name="ppq", bufs=3, space="PSUM"))
            pt = ph2a.enter_context(tc.tile_pool(name="ptq", bufs=3, space="PSUM"))
            load_xT = make_load_xT(stage, xtp, pt)
            Wq_sb = wqpool.tile([P, nck, d], f32, tag="wq")
            nc.sync.dma_start(Wq_sb[:].bitcast(f32r), Wqd[:].rearrange("(c p) n -> p c n", p=P).bitcast(f32r))
            xqT = load_xT(xq, 0, 2 * qb)
            for dc in range(nck):
                ps = pp.tile([P, 512], f32, tag="ps")
                for kc in range(nck):
                    nc.tensor.matmul(ps[:, :2 * qb],
                                     r32(Wq_sb[:, kc, dc * P:(dc + 1) * P]),
                                     r32(xqT[:, kc, :]),
                                     start=(kc == 0), stop=(kc == nck - 1))
                nc.vector.tensor_scalar_add(QT[:, dc, :].bitcast(f32r), ps[:, :2 * qb],
                                            biasq[:, dc:dc + 1])

        with ExitStack() as ph2b:
            wpool = ph2b.enter_context(tc.tile_pool(name="wpool", bufs=1))
            stage = ph2b.enter_context(tc.tile_pool(name="stage", bufs=3))
            xtp = ph2b.enter_context(tc.tile_pool(name="xtp", bufs=2))
            pp = ph2b.enter_context(tc.tile_pool(name="pp", bufs=3, space="PSUM"))
            pt = ph2b.enter_context(tc.tile_pool(name="pt", bufs=3, space="PSUM"))
            load_xT = make_load_xT(stage, xtp, pt)
            Wk_sb = wpool.tile([P, nck, d], f32, tag="wk")
            Wv_sb = wpool.tile([P, nck, d], f32, tag="wv")
            nc.sync.dma_start(Wk_sb[:].bitcast(f32r), Wkd[:].rearrange("(c p) n -> p c n", p=P).bitcast(f32r))
            nc.sync.dma_start(Wv_sb[:].bitcast(f32r), Wvd[:].rearrange("(c p) n -> p c n", p=P).bitcast(f32r))
            for g in range(s // 512):
                xkT = load_xT(xk, g * 512, 512)
                for dc in range(nck):
                    ps = pp.tile([P, 512], f32, tag="ps")
                    for kc in range(nck):
                        nc.tensor.matmul(ps, r32(Wk_sb[:, kc, dc * P:(dc + 1) * P]),
                                         r32(xkT[:, kc, :]),
                                         start=(kc == 0), stop=(kc == nck - 1))
                    nc.vector.tensor_scalar_add(KT[:, dc, g * 512:(g + 1) * 512].bitcast(f32r),
                                                ps, biask[:, dc:dc + 1])
                xvT = load_xT(xv, g * 512, 512)
                for sc in range(4):
                    kt = g * 4 + sc
                    for n0, nn in nsplits(d):
                        ps = pp.tile([P, 512], f32, tag="ps")
                        for kc in range(nck):
                            nc.tensor.matmul(ps[:, :nn],
                                             r32(xvT[:, kc, sc * P:(sc + 1) * P]),
                                             r32(Wv_sb[:, kc, n0:n0 + nn]),
                                             start=(kc == 0), stop=(kc == nck - 1))
                        nc.vector.tensor_copy(VA[:, kt, n0:n0 + nn].bitcast(f32r), ps[:, :nn])

        # ---- attention ----
        import concourse.bass as bass_mod
        with ExitStack() as ph3:
            mpool = ph3.enter_context(tc.tile_pool(name="mpool", bufs=1))
            epool = ph3.enter_context(tc.tile_pool(name="epool", bufs=4))
            rpool = ph3.enter_context(tc.tile_pool(name="rpool", bufs=3))
            lps = ph3.enter_context(tc.tile_pool(name="lps", bufs=3, space="PSUM"))
            aps = ph3.enter_context(tc.tile_pool(name="aps", bufs=1, space="PSUM"))
            mTs = mpool.tile([P, kt_hi, 2 * qb], f32)
            nc.sync.dma_start(mTs[:].bitcast(f32r), mT[:].rearrange("t p c -> p t c").bitcast(f32r))

            for h in range(nheads):
                hp, hc = (h % 2) * 64, h // 2
                ap_lo = aps.tile([64, qb], f32, tag="aplo")
                den_lo = aps.tile([64, qb], f32, tag="denlo")
                ap_hi = aps.tile([64, qb], f32, tag="aphi")
                den_hi = aps.tile([64, qb], f32, tag="denhi")
                # key tiles 0..kt_lo: shared by both q-blocks (N=512);
                # mask cols for block-hi are zeros there by construction
                for kt in range(kt_lo):
                    lg = lps.tile([P, 2 * qb], f32, tag="lg")
                    nc.tensor.matmul(
                        lg, r32(KT[hp:hp + 64, hc, kt * P:(kt + 1) * P]),
                        r32(QT[hp:hp + 64, hc, :]),
                        start=True, stop=False)
                    nc.tensor.matmul(lg, r32(negI), r32(mTs[:, kt, :]),
                                     start=False, stop=True)
                    E = epool.tile([P, 2 * qb], f32, tag="E")
                    nc.scalar.activation(E[:].bitcast(f32r), lg, Exp, scale=scale)
                    vh = r32(VA[:, kt, h * 64:(h + 1) * 64])
                    last = kt == kt_lo - 1
                    nc.tensor.matmul(ap_lo, vh, r32(E[:, 0:qb]),
                                     start=(kt == 0), stop=last)
                    nc.tensor.matmul(den_lo, r32(ones64[:]), r32(E[:, 0:qb]),
                                     start=(kt == 0), stop=last)
                    nc.tensor.matmul(ap_hi, vh, r32(E[:, qb:2 * qb]),
                                     start=(kt == 0), stop=False)
                    nc.tensor.matmul(den_hi, r32(ones64[:]), r32(E[:, qb:2 * qb]),
                                     start=(kt == 0), stop=False)
                rec = rpool.tile([64, qb], f32, tag="rec")
                nc.vector.reciprocal(rec, den_lo)
                nc.vector.tensor_mul(AT[hp:hp + 64, hc, 0:qb].bitcast(f32r),
                                     ap_lo, rec)
                # key tiles kt_lo..kt_hi: block-hi only
                for kt in range(kt_lo, kt_hi):
                    lg = lps.tile([P, 2 * qb], f32, tag="lg")
                    nc.tensor.matmul(
                        lg[:, 0:qb], r32(KT[hp:hp + 64, hc, kt * P:(kt + 1) * P]),
                        r32(QT[hp:hp + 64, hc, qb:2 * qb]),
                        start=True, stop=False)
                    nc.tensor.matmul(lg[:, 0:qb], r32(negI),
                                     r32(mTs[:, kt, qb:2 * qb]),
                                     start=False, stop=True)
                    E = epool.tile([P, 2 * qb], f32, tag="E")
                    nc.scalar.activation(E[:, 0:qb].bitcast(f32r), lg[:, 0:qb],
                                         Exp, scale=scale)
                    nc.tensor.matmul(ap_hi, r32(VA[:, kt, h * 64:(h + 1) * 64]),
                                     r32(E[:, 0:qb]),
                                     start=False, stop=(kt == kt_hi - 1))
                    nc.tensor.matmul(den_hi, r32(ones64[:]), r32(E[:, 0:qb]),
                                     start=False, stop=(kt == kt_hi - 1))
                rec2 = rpool.tile([64, qb], f32, tag="rec")
                nc.vector.reciprocal(rec2, den_hi)
                nc.vector.tensor_mul(AT[hp:hp + 64, hc, qb:2 * qb].bitcast(f32r),
                                     ap_hi, rec2)

        # ---- O-projection + bo' + relu ----
        with ExitStack() as ph4:
            wo_pool = ph4.enter_context(tc.tile_pool(name="wo", bufs=1))
            opool = ph4.enter_context(tc.tile_pool(name="opool", bufs=2))
            ops = ph4.enter_context(tc.tile_pool(name="ops", bufs=2, space="PSUM"))
            Wo_sb = wo_pool.tile([P, nck, d], f32)
            nc.sync.dma_start(Wo_sb[:].bitcast(f32r), Wod[:].rearrange("(c p) n -> p c n", p=P).bitcast(f32r))
            # bo' = bv @ Wo + bo
            for n0, nn in nsplits(d):
                ps = ops.tile([P, 512], f32, tag="pso")
                for kc in range(nck):
                    nc.tensor.matmul(ps[:1, :nn], r32(bvc_sb[:, kc:kc + 1]),
                                     r32(Wo_sb[:, kc, n0:n0 + nn]),
                                     start=(kc == 0), stop=(kc == nck - 1))
                nc.vector.tensor_add(boP[:, n0:n0 + nn].bitcast(f32r), ps[:1, :nn],
                                     bo_sb[:, n0:n0 + nn])
            for sub in range(2 * qb // P):
                osb = opool.tile([P, d], f32, tag="osb")
                for n0, nn in nsplits(d):
                    ps = ops.tile([P, 512], f32, tag="pso")
                    for kc in range(nck):
                        nc.tensor.matmul(ps[:, :nn],
                                         r32(AT[:, kc, sub * P:(sub + 1) * P]),
                                         r32(Wo_sb[:, kc, n0:n0 + nn]),
                                         start=(kc == 0), stop=False)
                    nc.tensor.matmul(ps[:, :nn], r32(ones1),
                                     r32(boP[:, n0:n0 + nn]),
                                     start=False, stop=True)
                    nc.scalar.activation(osb[:, n0:n0 + nn], ps[:, :nn], Relu)
                nc.sync.dma_start(out[sub * P:(sub + 1) * P, :], osb)

    nc.compile()
    names = dict(xq=xq.name, xk=xk.name, xv=xv.name, mT=mT.name,
                 Wq=Wqd.name, Wk=Wkd.name, Wv=Wvd.name, Wo=Wod.name,
                 bq=bqd.name, bk=bkd.name, bv=bvd.name, bo=bod.name,
                 out=out.name)
    return nc, names


def make_in_maps(names, q, k, v, mask, Wq, bq, Wk, bk, Wv, bv, Wo, bo,
                 s=S, d=D, n_cores=8):
    qb = s // 8
    kt_lo, kt_hi = s // 2 // 128, s // 128
    nck = d // 128
    mask2d = np.asarray(mask, np.float32).reshape(s, s)
    f = lambda x: np.ascontiguousarray(np.asarray(x), dtype=np.float32)
    in_maps = []
    for c in range(n_cores):
        b, j = c // 4, c % 4
        lo = slice(j * qb, (j + 1) * qb)
        hi = slice((7 - j) * qb, (8 - j) * qb)
        mTc = np.zeros((kt_hi, 128, 2 * qb), np.float32)
        for kt in range(kt_lo):
            mTc[kt, :, 0:qb] = mask2d[lo, kt * 128:(kt + 1) * 128].T
        for kt in range(kt_lo, kt_hi):
            mTc[kt, :, qb:2 * qb] = mask2d[hi, kt * 128:(kt + 1) * 128].T
        in_maps.append({
            names["xq"]: np.concatenate([f(q[b])[lo], f(q[b])[hi]], 0),
            names["xk"]: f(k[b]), names["xv"]: f(v[b]), names["mT"]: mTc,
            names["Wq"]: f(Wq), names["Wk"]: f(Wk), names["Wv"]: f(Wv),
            names["Wo"]: f(Wo),
            names["bq"]: f(bq).reshape(nck, 128),
            names["bk"]: f(bk).reshape(nck, 128),
            names["bv"]: f(bv).reshape(nck, 128),
            names["bo"]: f(bo).reshape(1, d),
        })
    return in_maps


def unshard(results, out_name, s=S, d=D):
    qb = s // 8
    full = np.zeros((B, s, d), np.float32)
    for c in range(len(results)):
        b, j = c // 4, c % 4
        oc = results[c][out_name]
        full[b, j * qb:(j + 1) * qb] = oc[:qb]
        full[b, (7 - j) * qb:(8 - j) * qb] = oc[qb:]
    return full


def kernel(q, k, v, mask, Wq, bq, Wk, bk, Wv, bv, Wo, bo):
    from concourse.bass_utils import run_bass_kernel_spmd
    if "prog" not in _prog_cache:
        _prog_cache["prog"] = build()
    nc, names = _prog_cache["prog"]
    in_maps = make_in_maps(names, q, k, v, mask, Wq, bq, Wk, bk, Wv, bv, Wo, bo)
    res = run_bass_kernel_spmd(nc, in_maps, core_ids=list(range(8)))
    return unshard(res.results, names["out"])


# revision 16
# speedup vs baseline: 1.0204x; 1.0051x over previous
"""Trainium2 Bass kernel: causal MHA (B=2,S=2048,D=768,H=12) on 8 NeuronCores.

Sharding: core c -> batch b=c//4, j=c%4; two q-blocks (t_lo=j, t_hi=7-j) of
S/8 rows each, for causal load balance. K/V projected fully per core.
Uniform SPMD program (one NEFF for all 8 cores; per-core data differs):
block-lo uses key tiles [0, KT_LO), mask-matmul on all of them; block-hi uses
key tiles [0, KT_HI), mask-matmul on [KT_LO, KT_HI). Masked/padded logits get
-1e9 added via a (-1e9*I) @ maskT accumulate matmul, so exp -> 0 exactly.
All data f32; matmuls run as float32r. Softmax denominator accumulates in its
own PSUM tile via a shared ones[128,64] stationary operand alongside the PV
matmuls; normalization is a per-partition DVE reciprocal+multiply.
"""
import sys
sys.path.insert(0, "/opt/trn_rl_repo")
from contextlib import ExitStack
import numpy as np

B, S, D, H, DK = 2, 2048, 768, 12, 64
_prog_cache = {}


def build(s=S, d=D):
    import concourse.bass as bass
    import concourse.mybir as mybir
    import concourse.tile as tile
    from concourse import bacc
    from concourse.masks import make_identity

    f32, f32r = mybir.dt.float32, mybir.dt.float32r
    P = 128
    nck = d // P              # D chunks (6)
    qb = s // 8               # q rows per block (256)
    kt_lo, kt_hi = s // 2 // P, s // P   # 8, 16
    nheads = d // 64
    scale = 1.0 / float(np.sqrt(d))
    Exp = mybir.ActivationFunctionType.Exp
    Relu = mybir.ActivationFunctionType.Relu

    nc = bacc.Bacc("TRN2", target_bir_lowering=False, debug=False)
    with tile.TileContext(nc) as tc, ExitStack() as top:
        dram = top.enter_context(tc.tile_pool(name="dram", bufs=1, space="DRAM"))
        xq = dram.tile([2 * qb, d], f32, kind="ExternalInput")
        xk = dram.tile([s, d], f32, kind="ExternalInput")
        xv = dram.tile([s, d], f32, kind="ExternalInput")
        mT = dram.tile([kt_hi, P, 2 * qb], f32, kind="ExternalInput")
        Wqd = dram.tile([d, d], f32, kind="ExternalInput")
        Wkd = dram.tile([d, d], f32, kind="ExternalInput")
        Wvd = dram.tile([d, d], f32, kind="ExternalInput")
        Wod = dram.tile([d, d], f32, kind="ExternalInput")
        bqd = dram.tile([nck, P], f32, kind="ExternalInput")
        bkd = dram.tile([nck, P], f32, kind="ExternalInput")
        bvd = dram.tile([nck, P], f32, kind="ExternalInput")
        bod = dram.tile([1, d], f32, kind="ExternalInput")
        out = dram.tile([2 * qb, d], f32, kind="ExternalOutput")

        persist = top.enter_context(tc.tile_pool(name="persist", bufs=1))
        KT = persist.tile([P, nck, s], f32)
        VA = persist.tile([P, s // P, d], f32)
        ones64 = persist.tile([P, 64], f32)
        QT = persist.tile([P, nck, 2 * qb], f32)
        AT = persist.tile([P, nck, 2 * qb], f32)
        ident = persist.tile([P, P], f32)
        negI = persist.tile([P, P], f32)
        biasq = persist.tile([P, nck], f32)
        biask = persist.tile([P, nck], f32)
        bvc_sb = persist.tile([P, nck], f32)
        bo_sb = persist.tile([1, d], f32)
        boP = persist.tile([1, d], f32)
        ones1 = persist.tile([1, P], f32)

        make_identity(nc, ident)
        ones_st = persist.tile([P, 64], f32)
        nc.scalar.mul(negI[:].bitcast(f32r), ident, -1e9)
        nc.vector.memset(ones_st, 1.0)
        ones1_st = persist.tile([1, P], f32)
        nc.vector.memset(ones1_st, 1.0)
        nc.vector.tensor_copy(ones1[:].bitcast(f32r), ones1_st)
        nc.vector.tensor_copy(ones64[:].bitcast(f32r), ones_st)
        nc.sync.dma_start(biasq, bqd[:].rearrange("a b -> b a"))
        nc.sync.dma_start(biask, bkd[:].rearrange("a b -> b a"))
        nc.sync.dma_start(bvc_sb[:].bitcast(f32r), bvd[:].rearrange("a b -> b a").bitcast(f32r))
        nc.sync.dma_start(bo_sb, bod)

        def r32(ap):
            return ap.bitcast(f32r)

        def nsplits(n):
            return [(i * 512, min(512, n - i * 512)) for i in range((n + 511) // 512)]

        def make_load_xT(stage, xtp, pt):
            def load_xT(xdram, row0, nrows):
                xT = xtp.tile([P, nck, nrows], f32, tag="xT")
                for sc in range(nrows // P):
                    xn = stage.tile([P, d], f32, tag="xn")
                    nc.sync.dma_start(xn, xdram[row0 + sc * P:row0 + (sc + 1) * P, :])
                    for dc in range(nck):
                        tp = pt.tile([P, P], f32, tag="tp")
                        nc.tensor.transpose(tp, xn[:, dc * P:(dc + 1) * P], ident)
                        nc.vector.tensor_copy(xT[:, dc, sc * P:(sc + 1) * P].bitcast(f32r), tp)
                return xT
            return load_xT

        with ExitStack() as ph2a:
            wqpool = ph2a.enter_context(tc.tile_pool(name="wqpool", bufs=1))
            stage = ph2a.enter_context(tc.tile_pool(name="stageq", bufs=3))
            xtp = ph2a.enter_context(tc.tile_pool(name="xtpq", bufs=2))
            pp = ph2a.enter_context(tc.tile_pool(name="ppq", bufs=3, space="PSUM"))
            pt = ph2a.enter_context(tc.tile_pool(name="ptq", bufs=3, space="PSUM"))
            load_xT = make_load_xT(stage, xtp, pt)
            Wq_sb = wqpool.tile([P, nck, d], f32, tag="wq")
            nc.sync.dma_start(Wq_sb[:].bitcast(f32r), Wqd[:].rearrange("(c p) n -> p c n", p=P).bitcast(f32r))
            xqT = load_xT(xq, 0, 2 * qb)
            for dc in range(nck):
                ps = pp.tile([P, 512], f32, tag="ps")
                for kc in range(nck):
                    nc.tensor.matmul(ps[:, :2 * qb],
                                     r32(Wq_sb[:, kc, dc * P:(dc + 1) * P]),
                                     r32(xqT[:, kc, :]),
                                     start=(kc == 0), stop=(kc == nck - 1))
                nc.vector.tensor_scalar_add(QT[:, dc, :].bitcast(f32r), ps[:, :2 * qb],
                                            biasq[:, dc:dc + 1])

        with ExitStack() as ph2b:
            wpool = ph2b.enter_context(tc.tile_pool(name="wpool", bufs=1))
            stage = ph2b.enter_context(tc.tile_pool(name="stage", bufs=3))
            xtp = ph2b.enter_context(tc.tile_pool(name="xtp", bufs=2))
            pp = ph2b.enter_context(tc.tile_pool(name="pp", bufs=3, space="PSUM"))
            pt = ph2b.enter_context(tc.tile_pool(name="pt", bufs=3, space="PSUM"))
            load_xT = make_load_xT(stage, xtp, pt)
            Wk_sb = wpool.tile([P, nck, d], f32, tag="wk")
            Wv_sb = wpool.tile([P, nck, d], f32, tag="wv")
            nc.sync.dma_start(Wk_sb[:].bitcast(f32r), Wkd[:].rearrange("(c p) n -> p c n", p=P).bitcast(f32r))
            nc.sync.dma_start(Wv_sb[:].bitcast(f32r), Wvd[:].rearrange("(c p) n -> p c n", p=P).bitcast(f32r))
            for g in range(s // 512):
                xkT = load_xT(xk, g * 512, 512)
                for dc in range(nck):
                    ps = pp.tile([P, 512], f32, tag="ps")
                    for kc in range(nck):
                        nc.tensor.matmul(ps, r32(Wk_sb[:, kc, dc * P:(dc + 1) * P]),
                                         r32(xkT[:, kc, :]),
                                         start=(kc == 0), stop=(kc == nck - 1))
                    nc.vector.tensor_scalar_add(KT[:, dc, g * 512:(g + 1) * 512].bitcast(f32r),
                                                ps, biask[:, dc:dc + 1])
                xvT = load_xT(xv, g * 512, 512)
                for sc in range(4):
                    kt = g * 4 + sc
                    for n0, nn in nsplits(d):
                        ps = pp.tile([P, 512], f32, tag="ps")
                        for kc in range(nck):
                            nc.tensor.matmul(ps[:, :nn],
                                             r32(xvT[:, kc, sc * P:(sc + 1) * P]),
                                             r32(Wv_sb[:, kc, n0:n0 + nn]),
                                             start=(kc == 0), stop=(kc == nck - 1))
                        nc.vector.tensor_copy(VA[:, kt, n0:n0 + nn].bitcast(f32r), ps[:, :nn])

        # ---- attention ----
        import concourse.bass as bass_mod
        with ExitStack() as ph3:
            mpool = ph3.enter_context(tc.tile_pool(name="mpool", bufs=1))
            epool = ph3.enter_context(tc.tile_pool(name="epool", bufs=4))
            rpool = ph3.enter_context(tc.tile_pool(name="rpool", bufs=3))
            lps = ph3.enter_context(tc.tile_pool(name="lps", bufs=3, space="PSUM"))
            aps = ph3.enter_context(tc.tile_pool(name="aps", bufs=1, space="PSUM"))
            mTs = mpool.tile([P, kt_hi, 2 * qb], f32)
            nc.sync.dma_start(mTs[:].bitcast(f32r), mT[:].rearrange("t p c -> p t c").bitcast(f32r))

            for h in range(nheads):
                hp, hc = (h % 2) * 64, h // 2
                ap_lo = aps.tile([64, qb], f32, tag="aplo")
                den_lo = aps.tile([64, qb], f32, tag="denlo")
                ap_hi = aps.tile([64, qb], f32, tag="aphi")
                den_hi = aps.tile([64, qb], f32, tag="denhi")
                # key tiles 0..kt_lo: shared by both q-blocks (N=512);
                # mask cols for block-hi are zeros there by construction
                for kt in range(kt_lo):
                    lg = lps.tile([P, 2 * qb], f32, tag="lg")
                    nc.tensor.matmul(
                        lg, r32(KT[hp:hp + 64, hc, kt * P:(kt + 1) * P]),
                        r32(QT[hp:hp + 64, hc, :]),
                        start=True, stop=True)
                    nc.tensor.matmul(lg[:, 0:qb], r32(negI),
                                     r32(mTs[:, kt, 0:qb]),
                                     start=False, stop=True,
                                     skip_group_check=True)
                    E = epool.tile([P, 2 * qb], f32, tag="E")
                    nc.scalar.activation(E[:].bitcast(f32r), lg, Exp, scale=scale)
                    vh = r32(VA[:, kt, h * 64:(h + 1) * 64])
                    last = kt == kt_lo - 1
                    nc.tensor.matmul(ap_lo, vh, r32(E[:, 0:qb]),
                                     start=(kt == 0), stop=last)
                    nc.tensor.matmul(den_lo, r32(ones64[:]), r32(E[:, 0:qb]),
                                     start=(kt == 0), stop=last)
                    nc.tensor.matmul(ap_hi, vh, r32(E[:, qb:2 * qb]),
                                     start=(kt == 0), stop=False)
                    nc.tensor.matmul(den_hi, r32(ones64[:]), r32(E[:, qb:2 * qb]),
                                     start=(kt == 0), stop=False)
                rec = rpool.tile([64, qb], f32, tag="rec")
                nc.vector.reciprocal(rec, den_lo)
                nc.vector.tensor_mul(AT[hp:hp + 64, hc, 0:qb].bitcast(f32r),
                                     ap_lo, rec)
                # key tiles kt_lo..kt_hi: block-hi only
                for kt in range(kt_lo, kt_hi):
                    lg = lps.tile([P, 2 * qb], f32, tag="lg")
                    nc.tensor.matmul(
                        lg[:, 0:qb], r32(KT[hp:hp + 64, hc, kt * P:(kt + 1) * P]),
                        r32(QT[hp:hp + 64, hc, qb:2 * qb]),
                        start=True, stop=False)
                    nc.tensor.matmul(lg[:, 0:qb], r32(negI),
                                     r32(mTs[:, kt, qb:2 * qb]),
                                     start=False, stop=True)
                    E = epool.tile([P, 2 * qb], f32, tag="E")
                    nc.scalar.activation(E[:, 0:qb].bitcast(f32r), lg[:, 0:qb],
                                         Exp, scale=scale)
                    nc.tensor.matmul(ap_hi, r32(VA[:, kt, h * 64:(h + 1) * 64]),
                                     r32(E[:, 0:qb]),
                                     start=False, stop=(kt == kt_hi - 1))
                    nc.tensor.matmul(den_hi, r32(ones64[:]), r32(E[:, 0:qb]),
                                     start=False, stop=(kt == kt_hi - 1))
                rec2 = rpool.tile([64, qb], f32, tag="rec")
                nc.vector.reciprocal(rec2, den_hi)
                nc.vector.tensor_mul(AT[hp:hp + 64, hc, qb:2 * qb].bitcast(f32r),
                                     ap_hi, rec2)

        # ---- O-projection + bo' + relu ----
        with ExitStack() as ph4:
            wo_pool = ph4.enter_context(tc.tile_pool(name="wo", bufs=1))
            opool = ph4.enter_context(tc.tile_pool(name="opool", bufs=2))
            ops = ph4.enter_context(tc.tile_pool(name="ops", bufs=2, space="PSUM"))
            Wo_sb = wo_pool.tile([P, nck, d], f32)
            nc.sync.dma_start(Wo_sb[:].bitcast(f32r), Wod[:].rearrange("(c p) n -> p c n", p=P).bitcast(f32r))
            # bo' = bv @ Wo + bo
            for n0, nn in nsplits(d):
                ps = ops.tile([P, 512], f32, tag="pso")
                for kc in range(nck):
                    nc.tensor.matmul(ps[:1, :nn], r32(bvc_sb[:, kc:kc + 1]),
                                     r32(Wo_sb[:, kc, n0:n0 + nn]),
                                     start=(kc == 0), stop=(kc == nck - 1))
                nc.vector.tensor_add(boP[:, n0:n0 + nn].bitcast(f32r), ps[:1, :nn],
                                     bo_sb[:, n0:n0 + nn])
            for sub in range(2 * qb // P):
                osb = opool.tile([P, d], f32, tag="osb")
                for n0, nn in nsplits(d):
                    ps = ops.tile([P, 512], f32, tag="pso")
                    for kc in range(nck):
                        nc.tensor.matmul(ps[:, :nn],
                                         r32(AT[:, kc, sub * P:(sub + 1) * P]),
                                         r32(Wo_sb[:, kc, n0:n0 + nn]),
                                         start=(kc == 0), stop=False)
                    nc.tensor.matmul(ps[:, :nn], r32(ones1),
                                     r32(boP[:, n0:n0 + nn]),
                                     start=False, stop=True)
                    nc.scalar.activation(osb[:, n0:n0 + nn], ps[:, :nn], Relu)
                nc.sync.dma_start(out[sub * P:(sub + 1) * P, :], osb)

    nc.compile()
    names = dict(xq=xq.name, xk=xk.name, xv=xv.name, mT=mT.name,
                 Wq=Wqd.name, Wk=Wkd.name, Wv=Wvd.name, Wo=Wod.name,
                 bq=bqd.name, bk=bkd.name, bv=bvd.name, bo=bod.name,
                 out=out.name)
    return nc, names


def make_in_maps(names, q, k, v, mask, Wq, bq, Wk, bk, Wv, bv, Wo, bo,
                 s=S, d=D, n_cores=8):
    qb = s // 8
    kt_lo, kt_hi = s // 2 // 128, s // 128
    nck = d // 128
    mask2d = np.asarray(mask, np.float32).reshape(s, s)
    f = lambda x: np.ascontiguousarray(np.asarray(x), dtype=np.float32)
    in_maps = []
    for c in range(n_cores):
        b, j = c // 4, c % 4
        lo = slice(j * qb, (j + 1) * qb)
        hi = slice((7 - j) * qb, (8 - j) * qb)
        mTc = np.zeros((kt_hi, 128, 2 * qb), np.float32)
        for kt in range(kt_lo):
            mTc[kt, :, 0:qb] = mask2d[lo, kt * 128:(kt + 1) * 128].T
        for kt in range(kt_lo, kt_hi):
            mTc[kt, :, qb:2 * qb] = mask2d[hi, kt * 128:(kt + 1) * 128].T
        in_maps.append({
            names["xq"]: np.concatenate([f(q[b])[lo], f(q[b])[hi]], 0),
            names["xk"]: f(k[b]), names["xv"]: f(v[b]), names["mT"]: mTc,
            names["Wq"]: f(Wq), names["Wk"]: f(Wk), names["Wv"]: f(Wv),
            names["Wo"]: f(Wo),
            names["bq"]: f(bq).reshape(nck, 128),
            names["bk"]: f(bk).reshape(nck, 128),
            names["bv"]: f(bv).reshape(nck, 128),
            names["bo"]: f(bo).reshape(1, d),
        })
    return in_maps


def unshard(results, out_name, s=S, d=D):
    qb = s // 8
    full = np.zeros((B, s, d), np.float32)
    for c in range(len(results)):
        b, j = c // 4, c % 4
        oc = results[c][out_name]
        full[b, j * qb:(j + 1) * qb] = oc[:qb]
        full[b, (7 - j) * qb:(8 - j) * qb] = oc[qb:]
    return full


def kernel(q, k, v, mask, Wq, bq, Wk, bk, Wv, bv, Wo, bo):
    from concourse.bass_utils import run_bass_kernel_spmd
    if "prog" not in _prog_cache:
        _prog_cache["prog"] = build()
    nc, names = _prog_cache["prog"]
    in_maps = make_in_maps(names, q, k, v, mask, Wq, bq, Wk, bk, Wv, bv, Wo, bo)
    res = run_bass_kernel_spmd(nc, in_maps, core_ids=list(range(8)))
    return unshard(res.results, names["out"])
